# revision 2
# baseline (speedup 1.0000x reference)
"""Trainium2 8-core kernel for MemoryEfficientAttention (bf16 rewrite).

Full MHA layer: Q/K/V projections + exact softmax attention + out-projection
for [B=4, S=2048, D=1024], H=16, dk=64. Core c = (batch c//2, head-half c%2).
Host sums the two partial out-projections per batch and adds the bias.

Per-core dataflow (bf16 operands, fp32 PSUM):
  xT[dt]    = DMA-transpose of x[:, dt*128:+128]        [128, 2048] (xbar)
  V_aug[kt] = x_v @ Wv + bv, 65-stride per head w/ ones  [128, 8*65]
  KT[jt]    = (x_k @ Wk + bk).T packed 2 heads/tile      [128, 2048]
  QT[jt]    = (x_q @ Wq*s + bq*s).T packed               [128, 2048]
  per (qg 512-q group, jt head pair), kt = 0..15 software-pipelined:
    sT  = K_hh.T @ QT_hh   2x 64-contract mms (tile rows 0/64) [128k, 1024q]
    eT  = exp(sT)          ACT, one [128,1024] instr -> bf16 SBUF
    av  += V_aug_h.T @ eT_hh  65-row out (vals + denom row 64), 2 PSUM tiles
  finalize: recip(denom) -> PE ones-bcast to 64 rows -> DVE mult -> oc[jt]
    (hh1 written to partitions 64:128 via DVE partition shift)
  y[qt] = sum_jt oc[jt].T @ Wo  interleaved into later attention groups
"""

import numpy as np

import concourse.bass as bass
import concourse.mybir as mybir
import concourse.tile as tile
from concourse import bacc

B, S, D, H, DK = 4, 2048, 1024, 16, 64
NCORES = 8
HPC = H // 2          # heads per core
DH = HPC * DK         # 512 projection dims per core
NJT = 4               # head-pair tiles
NDT = 8               # d tiles of 128
NKT = 16              # k tiles of 128
NQG = 4               # q groups of 512
NQT = 16              # q tiles of 128
VW = HPC * (DK + 1)   # V_aug width: 8 heads x (64 vals + ones col)
F32 = mybir.dt.float32
BF16 = mybir.dt.bfloat16
EXP = mybir.ActivationFunctionType.Exp


def _emit(nc, tc, ctx):
    # host supplies x pre-transposed: xT [D, S]
    xq = nc.dram_tensor("xq", [D, S], BF16, kind="ExternalInput").ap()
    xk = nc.dram_tensor("xk", [D, S], BF16, kind="ExternalInput").ap()
    xv = nc.dram_tensor("xv", [D, S], BF16, kind="ExternalInput").ap()
    wq = nc.dram_tensor("wq", [D, DH], BF16, kind="ExternalInput").ap()
    wk = nc.dram_tensor("wk", [D, DH], BF16, kind="ExternalInput").ap()
    wv = nc.dram_tensor("wv", [D, DH], BF16, kind="ExternalInput").ap()
    wo = nc.dram_tensor("wo", [DH, D], BF16, kind="ExternalInput").ap()
    bq = nc.dram_tensor("bq", [DH], F32, kind="ExternalInput").ap()
    bk = nc.dram_tensor("bk", [DH], F32, kind="ExternalInput").ap()
    bv = nc.dram_tensor("bv", [DH], F32, kind="ExternalInput").ap()
    y = nc.dram_tensor("y", [S, D], BF16, kind="ExternalOutput").ap()

    consts = ctx.enter_context(tc.tile_pool(name="consts", bufs=1))
    wpool = ctx.enter_context(tc.tile_pool(name="weights", bufs=2))
    xtp = ctx.enter_context(tc.tile_pool(name="xt", bufs=2))
    projp = ctx.enter_context(tc.tile_pool(name="proj", bufs=1))
    expp = ctx.enter_context(tc.tile_pool(name="expt", bufs=3))
    smalls = ctx.enter_context(tc.tile_pool(name="smalls", bufs=2))
    ystage = ctx.enter_context(tc.tile_pool(name="ystage", bufs=2))
    psum = ctx.enter_context(tc.tile_pool(name="psum", bufs=1, space="PSUM"))

    # PSUM: "s" scores [128,1024] x2 (4 banks), "b" attnV accum x2 (2 banks),
    # "p" proj/outproj/bcast x2 (2 banks)
    def ps_s(name):
        return psum.tile([128, 1024], F32, tag="s", name=name, bufs=2)

    def ps_b(name):
        return psum.tile([128, 512], F32, tag="b", name=name, bufs=2)

    def ps_p(name):
        return psum.tile([128, 512], F32, tag="p", name=name, bufs=2)

    ones = consts.tile([128, 64], BF16)
    nc.vector.memset(ones, 1.0)
    bq_sb = consts.tile([128, NJT], F32)
    nc.sync.dma_start(out=bq_sb, in_=bq.rearrange("(a p) -> p a", p=128))
    bk_sb = consts.tile([128, NJT], F32)
    nc.sync.dma_start(out=bk_sb, in_=bk.rearrange("(a p) -> p a", p=128))
    bv_sb = consts.tile([128, DH], F32)
    nc.sync.dma_start(
        out=bv_sb,
        in_=bass.AP(tensor=bv.tensor, offset=bv.offset, ap=[[0, 128], [1, DH]]),
    )

    qt_t = [projp.tile([128, S], BF16, tag=f"q{jt}", name=f"qT{jt}")
            for jt in range(NJT)]
    kt_t = [projp.tile([128, S], BF16, tag=f"k{jt}", name=f"kT{jt}")
            for jt in range(NJT)]
    v_t = [projp.tile([128, VW], BF16, tag=f"v{kt}", name=f"v{kt}")
           for kt in range(NKT)]
    oc_t = [projp.tile([128, S], BF16, tag=f"oc{jt}", name=f"oc{jt}")
            for jt in range(NJT)]

    # ones column per head in V_aug (written once)
    for kt in range(NKT):
        vv = v_t[kt].rearrange("p (h c) -> p h c", c=DK + 1)
        nc.vector.memset(vv[:, :, DK:DK + 1], 1.0)

    def load_xt(x_dram):
        xts = [xtp.tile([128, S], BF16, tag=f"xt{dt}", name=f"xt{dt}", bufs=2)
               for dt in range(NDT)]
        for dt in range(NDT):
            nc.sync.dma_start(out=xts[dt], in_=x_dram[dt * 128:(dt + 1) * 128, :])
        return xts

    # ---- V projection: natural [k, j] layout with ones column per head ----
    w_v = wpool.tile([128, NDT, DH], BF16, tag="w", name="w_v")
    nc.sync.dma_start(out=w_v, in_=wv.rearrange("(n p) j -> p n j", p=128))
    xtv = load_xt(xv)
    for kt in range(NKT):
        pv = ps_p(f"pv{kt}")
        for dt in range(NDT):
            nc.tensor.matmul(
                pv[:],
                lhsT=xtv[dt][:, kt * 128:(kt + 1) * 128],
                rhs=w_v[:, dt, :],
                start=(dt == 0),
                stop=(dt == NDT - 1),
            )
        vv = v_t[kt].rearrange("p (h c) -> p h c", c=DK + 1)
        nc.vector.tensor_add(
            out=vv[:, :, 0:DK],
            in0=pv.rearrange("p (h d) -> p h d", h=HPC),
            in1=bv_sb.rearrange("p (h d) -> p h d", h=HPC),
        )

    # ---- K/Q projections: transposed [j, q] packed 2 heads per tile ----
    w_k = wpool.tile([128, NDT, DH], BF16, tag="w", name="w_k")
    nc.sync.dma_start(out=w_k, in_=wk.rearrange("(n p) j -> p n j", p=128))
    w_q = wpool.tile([128, NDT, DH], BF16, tag="wq", name="w_q")
    nc.sync.dma_start(out=w_q, in_=wq.rearrange("(n p) j -> p n j", p=128))
    wo_sb = wpool.tile([128, NJT, D], BF16, tag="wo", name="w_o")
    nc.sync.dma_start(out=wo_sb, in_=wo.rearrange("(n p) j -> p n j", p=128))

    xtk = load_xt(xk)

    def kproj(jt, qg):
        pq = ps_p(f"pk{jt}{qg}")
        for dt in range(NDT):
            nc.tensor.matmul(
                pq[:],
                lhsT=w_k[:, dt, jt * 128:(jt + 1) * 128],
                rhs=xtk[dt][:, qg * 512:(qg + 1) * 512],
                start=(dt == 0),
                stop=(dt == NDT - 1),
            )
        nc.vector.tensor_scalar_add(
            out=kt_t[jt][:, qg * 512:(qg + 1) * 512],
            in0=pq[:],
            scalar1=bk_sb[:, jt:jt + 1],
        )

    for jt in range(NJT):
        for qg in range(NQG):
            kproj(jt, qg)

    xtq = load_xt(xq)

    def qproj_part(pq, jt, qg, dts):
        for dt in dts:
            nc.tensor.matmul(
                pq[:],
                lhsT=w_q[:, dt, jt * 128:(jt + 1) * 128],
                rhs=xtq[dt][:, qg * 512:(qg + 1) * 512],
                start=(dt == 0),
                stop=(dt == NDT - 1),
            )
        if dts[-1] == NDT - 1:
            nc.vector.tensor_scalar_add(
                out=qt_t[jt][:, qg * 512:(qg + 1) * 512],
                in0=pq[:],
                scalar1=bq_sb[:, jt:jt + 1],
            )

    def qproj(jt, qg):
        pq = ps_p(f"pq{jt}{qg}")
        qproj_part(pq, jt, qg, range(NDT))

    for jt in range(NJT):
        qproj(jt, 0)

    # ---- out-projection of one 128-row q tile (two emission halves) ----
    def outproj_part(py, qt, jts):
        for jt in jts:
            for nb in range(2):
                nc.tensor.matmul(
                    py[nb][:],
                    lhsT=oc_t[jt][:, qt * 128:(qt + 1) * 128],
                    rhs=wo_sb[:, jt, nb * 512:(nb + 1) * 512],
                    start=(jt == 0),
                    stop=(jt == NJT - 1),
                )
        if jts[-1] == NJT - 1:
            ys = ystage.tile([128, 1024], BF16, tag="y", name="ys", bufs=2)
            for nb in range(2):
                nc.vector.tensor_copy(out=ys[:, nb * 512:(nb + 1) * 512],
                                      in_=py[nb][:])
            nc.sync.dma_start(out=y[qt * 128:(qt + 1) * 128, :], in_=ys[:])

    def outproj(qt):
        py = [ps_p(f"py{qt}{nb}") for nb in range(2)]
        outproj_part(py, qt, (0, 1, 2, 3))

    # ---- attention ----
    # finalize part 2: recip + PE row-broadcast + normalize into oc
    def finalize2(jt, qg, avsb_a, avsb_b):
        # reciprocal_approx_fast mishandles partition-sliced APs; run it on the
        # full tile (rows other than the denominator row 64 are never read)
        rcf = smalls.tile([128, 1024], F32, tag="rcf", name="rcf", bufs=1)
        nc.vector.reciprocal_approx_fast(rcf[:, 0:512], avsb_a[:, :])
        nc.vector.reciprocal_approx_fast(rcf[:, 512:1024], avsb_b[:, :])
        rc = smalls.tile([128, 1024], BF16, tag="rc", name="rc", bufs=2)
        nc.vector.tensor_copy(out=rc[64:65, :], in_=rcf[64:65, :])
        rbc_a = ps_p(f"ra{jt}{qg}")
        nc.tensor.matmul(rbc_a[0:64, :], lhsT=ones[64:65, 0:64],
                         rhs=rc[64:65, 0:512], start=True, stop=True)
        rbc_b = ps_p(f"rb{jt}{qg}")
        nc.tensor.matmul(rbc_b[0:64, :], lhsT=ones[64:65, 0:64],
                         rhs=rc[64:65, 512:1024], start=True, stop=True)
        qs = slice(qg * 512, (qg + 1) * 512)
        nc.vector.tensor_mul(out=oc_t[jt][0:64, qs], in0=avsb_a[0:64, :],
                             in1=rbc_a[0:64, :])
        nc.vector.tensor_mul(out=oc_t[jt][64:128, qs], in0=avsb_b[0:64, :],
                             in1=rbc_b[0:64, :])

    pending2 = []

    def attention_group(jt, qg, slots):
        av_a = ps_b(f"ava{jt}{qg}")
        av_b = ps_b(f"avb{jt}{qg}")
        pss = {}
        ets = {}

        def scores(kt):
            ps = ps_s(f"ss{kt % 2}")
            for hh in range(2):
                nc.tensor.matmul(
                    ps[:, hh * 512:(hh + 1) * 512],
                    lhsT=kt_t[jt][hh * 64:(hh + 1) * 64, kt * 128:(kt + 1) * 128],
                    rhs=qt_t[jt][hh * 64:(hh + 1) * 64, qg * 512:(qg + 1) * 512],
                    start=True,
                    stop=True,
                )
            pss[kt] = ps

        def expk(kt):
            e = expp.tile([128, 1024], BF16, tag="e", name="eT", bufs=3)
            nc.scalar.activation(e[:], pss.pop(kt)[:], EXP)
            ets[kt] = e

        def attnv(kt):
            e = ets.pop(kt)
            for hh, av in ((0, av_a), (1, av_b)):
                h = 2 * jt + hh
                nc.tensor.matmul(
                    av[0:DK + 1, :],
                    lhsT=v_t[kt][:, h * (DK + 1):(h + 1) * (DK + 1)],
                    rhs=e[:, hh * 512:(hh + 1) * 512],
                    start=(kt == 0),
                    stop=(kt == NKT - 1),
                )

        scores(0)
        expk(0)
        scores(1)
        expk(1)
        for kt in range(NKT):
            if kt == 2 and pending2:
                finalize2(*pending2.pop(0))
            cb = slots.get(kt)
            if cb is not None:
                cb()
            attnv(kt)
            if kt + 2 < NKT:
                scores(kt + 2)
                expk(kt + 2)

        # finalize part 1: evict accumulators (frees "b" psum for next group)
        avsb_a = smalls.tile([128, 512], F32, tag="avsb", name="avsb_a", bufs=4)
        nc.vector.tensor_copy(out=avsb_a[0:DK + 1, :], in_=av_a[0:DK + 1, :])
        avsb_b = smalls.tile([128, 512], F32, tag="avsb", name="avsb_b", bufs=4)
        nc.vector.tensor_copy(out=avsb_b[0:DK + 1, :], in_=av_b[0:DK + 1, :])
        pending2.append((jt, qg, avsb_a, avsb_b))

    for qg in range(NQG):
        for jt in range(NJT):
            slots = {}
            if qg + 1 < NQG:
                slots[6] = (lambda j=jt, q=qg + 1: qproj(j, q))
            if qg >= 1:
                slots[13] = (lambda t=(qg - 1) * 4 + jt: outproj(t))
            attention_group(jt, qg, slots)

    while pending2:
        finalize2(*pending2.pop(0))
    for qt in range(12, NQT):
        outproj(qt)


_CACHE = {}


def _build():
    if "nc" in _CACHE:
        return _CACHE["nc"]
    from contextlib import ExitStack

    nc = bacc.Bacc("TRN2", target_bir_lowering=False, debug=False,
                   num_devices=NCORES)
    with tile.TileContext(nc) as tc:
        with ExitStack() as ctx:
            _emit(nc, tc, ctx)
    nc.compile()
    _CACHE["nc"] = nc
    return nc


def make_in_maps(query, key, value, Wq, bq, Wk, bk, Wv, bv, Wo, bo):
    import ml_dtypes
    bf16 = ml_dtypes.bfloat16
    arrs = [np.asarray(a, dtype=np.float32)
            for a in (query, key, value, Wq, bq, Wk, bk, Wv, bv, Wo, bo)]
    query, key, value, Wq, bq, Wk, bk, Wv, bv, Wo, bo = arrs
    scale = np.float32(1.0 / np.sqrt(DK))
    xqb = [np.ascontiguousarray(query[b].T).astype(bf16) for b in range(B)]
    xkb = [np.ascontiguousarray(key[b].T).astype(bf16) for b in range(B)]
    xvb = [np.ascontiguousarray(value[b].T).astype(bf16) for b in range(B)]
    in_maps = []
    for c in range(NCORES):
        b, hh = divmod(c, 2)
        js = slice(hh * DH, (hh + 1) * DH)
        in_maps.append({
            "xq": xqb[b],
            "xk": xkb[b],
            "xv": xvb[b],
            "wq": np.ascontiguousarray(Wq[:, js] * scale).astype(bf16),
            "bq": np.ascontiguousarray(bq[js] * scale),
            "wk": np.ascontiguousarray(Wk[:, js]).astype(bf16),
            "bk": np.ascontiguousarray(bk[js]),
            "wv": np.ascontiguousarray(Wv[:, js]).astype(bf16),
            "bv": np.ascontiguousarray(bv[js]),
            "wo": np.ascontiguousarray(Wo[js, :]).astype(bf16),
        })
    return in_maps


LAST_RESULTS = None


def kernel(query, key, value, Wq, bq, Wk, bk, Wv, bv, Wo, bo):
    global LAST_RESULTS
    import os
    from concourse.bass_utils import run_bass_kernel_spmd

    nc = _build()
    in_maps = make_in_maps(query, key, value, Wq, bq, Wk, bk, Wv, bv, Wo, bo)
    trace = bool(int(os.environ.get("KERNEL_TRACE", "0")))
    res = run_bass_kernel_spmd(nc, in_maps, list(range(NCORES)), trace=trace)
    LAST_RESULTS = res
    bo32 = np.asarray(bo, dtype=np.float32)
    out = np.empty((B, S, D), dtype=np.float32)
    for b in range(B):
        out[b] = (res.results[2 * b]["y"].astype(np.float32)
                  + res.results[2 * b + 1]["y"].astype(np.float32) + bo32)
    return out
